# revision 1
# baseline (speedup 1.0000x reference)
"""GCN layer (Chebyshev) Trainium2 kernel, 8-core SPMD.

out = BatchNorm2d(einsum('kmn,bcmt,kco->bont', cheb, relu(x), theta))

Sharding: data-parallel over batch B=16 -> 2 batches/core, cheb+theta
replicated. BN batch stats are combined with a tiny (256 B) AllReduce.

Per-core device program:
  stage T: relu(x) on ACT, then theta contraction as 96 matmuls
           lhsT = xr[(t4,c32), m128] (stationary), rhs = block-diag theta
           [128, (k,t',o)=384] -> w[(k,m) x (b,t,o)] resident in SBUF.
  stage M: per 128-wide n-tile, accumulate 48 chunk matmuls
           psum[n, (b,t,o)] += cheb[(k,m)chunk, n].T @ w[chunk] (bf16, f32 acc).
  stage S: per-channel sum / sumsq partials (ACT squares, DVE reduces),
           PE ones-vector column reduce, AllReduce, 1/sqrt -> scale/bias
           rows, broadcast, normalize in SBUF, DMA out as [b, n, t, o].
Host: input transpose/cast + output transpose are data movement only.
"""

import contextlib
import os

import numpy as np
import ml_dtypes

import concourse.bass as bass
import concourse.bacc as bacc
import concourse.tile as tile
import concourse.mybir as mybir
import concourse.bass_utils as bass_utils

N_CORES = 8
B, C, N, T = 16, 32, 2048, 12
K, O = 3, 32
BL = B // N_CORES            # 2 batches per core
TQ = T // 4                  # 3 quads of 4 timesteps
MC = N // 128                # 16 m-chunks
NT = N // 128                # 16 n-tiles
CH = K * MC                  # 48 contraction chunks of 128
F = BL * T * O               # 768 free columns (b, t, o)
BN_EPS = 1e-5
BN_COUNT = B * N * T         # stats population per channel

BF16 = mybir.dt.bfloat16
F32 = mybir.dt.float32
AF = mybir.ActivationFunctionType
ALL_STAGES = frozenset({"theta", "big", "stats", "norm"})

_CACHE = {}


def _build(single_core=False, stages=ALL_STAGES, loop_reps=0, nocc=False):
    nc = bacc.Bacc("TRN2", target_bir_lowering=False, debug=False,
                   num_devices=1 if single_core else N_CORES)
    xt = nc.dram_tensor("xt", [BL, T, C, N], BF16, kind="ExternalInput")
    cheb = nc.dram_tensor("cheb", [K, N, N], BF16, kind="ExternalInput")
    bd = nc.dram_tensor("bd", [128, K * 4 * O], BF16, kind="ExternalInput")
    out_d = nc.dram_tensor("out", [BL, N, T, O], F32, kind="ExternalOutput")

    with tile.TileContext(nc) as tc:
        with (
            tc.tile_pool(name="const", bufs=1) as constp,
            tc.tile_pool(name="xin", bufs=2) as xin,
            tc.tile_pool(name="wall", bufs=1) as wallp,
            tc.tile_pool(name="chebp", bufs=2) as chebp,
            tc.tile_pool(name="outp", bufs=1) as outp,
            tc.tile_pool(name="small", bufs=1) as small,
            tc.tile_pool(name="scratch", bufs=2) as scratch,
            tc.tile_pool(name="psw", bufs=3, space="PSUM") as psw,
            tc.tile_pool(name="psb", bufs=2, space="PSUM") as psb,
            tc.tile_pool(name="dram", bufs=2, space="DRAM") as dram,
        ):
            bd_s = constp.tile([128, K * 4 * O], BF16)
            nc.sync.dma_start(bd_s[:], bd[:])

            # w[(k,mc) chunks x (b,t,o)]
            w_all = wallp.tile([128, CH * F], BF16)
            wv = w_all[:].rearrange("p (k mc b t o) -> p k mc b t o",
                                    k=K, mc=MC, b=BL, t=T, o=O)
            if "theta" not in stages and "big" in stages:
                nc.vector.memset(w_all[:], 0.5)
            out_sb = outp.tile([128, NT * F], BF16)
            if "big" not in stages:
                nc.vector.memset(out_sb[:], 0.25)
            stats = small.tile([128, 2 * O], F32)
            nc.vector.memset(stats[:], 0.0)
            cheb_v = cheb[:].rearrange("k (mc p) n -> p (k mc) n", p=128)

            loop_cm = tc.For_i(0, loop_reps, 1) if loop_reps \
                else contextlib.nullcontext()
            with loop_cm:
                # ---- stage T: load + relu ----
                xr_all = xin.tile([128, BL * TQ * N], BF16, tag="xr_all")
                for b in range(BL):
                    for tq in range(TQ):
                        xtl = xin.tile([128, N], BF16, tag="xtl")
                        src = xt[b, tq * 4:(tq + 1) * 4].rearrange(
                            "t c m -> (t c) m")
                        nc.sync.dma_start(xtl[:], src)
                        if "theta" in stages:
                            q = (b * TQ + tq) * N
                            nc.scalar.activation(xr_all[:, q:q + N], xtl[:],
                                                 AF.Relu)

                # theta contraction, mc-outer so w chunks finish early
                if "theta" in stages:
                    cnt = 0
                    for mc in range(MC):
                        for b in range(BL):
                            for tq in range(TQ):
                                q = (b * TQ + tq) * N
                                pw = psw.tile([128, K * 4 * O], F32, tag="pw")
                                nc.tensor.matmul(
                                    pw[:],
                                    xr_all[:, q + mc * 128:q + (mc + 1) * 128],
                                    bd_s[:], start=True, stop=True)
                                # pw free = (k, t', o); dest (k, t, o) slice
                                dest = wv[:, :, mc, b,
                                          tq * 4:(tq + 1) * 4, :]
                                srcv = pw[:].rearrange(
                                    "p (k t o) -> p k t o", k=K, t=4, o=O)
                                if cnt % 2 == 0:
                                    nc.vector.tensor_copy(dest, srcv)
                                else:
                                    nc.scalar.copy(dest, srcv)
                                cnt += 1

                # ---- stage M: big matmul + stats partials ----
                for nt in range(NT):
                    cb = chebp.tile([128, CH, 128], BF16, tag="cb")
                    nc.sync.dma_start(cb[:],
                                      cheb_v[:, :, nt * 128:(nt + 1) * 128])
                    if "big" in stages:
                        po0 = psb.tile([128, F // 2], F32, tag="po0")
                        po1 = psb.tile([128, F // 2], F32, tag="po1")
                        n_ch = 0
                        for mc in range(MC):
                            for k in range(K):
                                ch = k * MC + mc
                                lhs = cb[:, ch, :]
                                first = n_ch == 0
                                last = n_ch == CH - 1
                                nc.tensor.matmul(
                                    po0[:], lhs,
                                    w_all[:, ch * F:ch * F + F // 2],
                                    start=first, stop=last)
                                nc.tensor.matmul(
                                    po1[:], lhs,
                                    w_all[:, ch * F + F // 2:(ch + 1) * F],
                                    start=first, stop=last)
                                n_ch += 1
                        sl = out_sb[:, nt * F:(nt + 1) * F]
                        nc.scalar.copy(sl[:, 0:F // 2], po0[:])
                        nc.scalar.copy(sl[:, F // 2:F], po1[:])
                    if "stats" in stages and "big" in stages:
                        sq = scratch.tile([128, F], BF16, tag="sq")
                        nc.scalar.activation(sq[:, 0:F // 2], po0[:],
                                             AF.Square)
                        nc.scalar.activation(sq[:, F // 2:F], po1[:],
                                             AF.Square)
                        tmp_s = scratch.tile([128, O], F32, tag="tmp_s")
                        tmp_q = scratch.tile([128, O], F32, tag="tmp_q")
                        nc.vector.reduce_sum(
                            tmp_s[:],
                            sl.rearrange("p (b t o) -> p o b t",
                                         b=BL, t=T, o=O),
                            axis=mybir.AxisListType.XY)
                        nc.vector.reduce_sum(
                            tmp_q[:],
                            sq[:].rearrange("p (b t o) -> p o b t",
                                            b=BL, t=T, o=O),
                            axis=mybir.AxisListType.XY)
                        nc.vector.tensor_add(stats[:, 0:O], stats[:, 0:O],
                                             tmp_s[:])
                        nc.vector.tensor_add(stats[:, O:2 * O],
                                             stats[:, O:2 * O], tmp_q[:])

            # ---- stage S: finalize stats, AllReduce, normalize ----
            do_stats = "stats" in stages
            if do_stats:
                ones = small.tile([128, 1], F32)
                nc.vector.memset(ones[:], 1.0)
                ps_st = psw.tile([1, 2 * O], F32, tag="pw")
                nc.tensor.matmul(ps_st[:], ones[:], stats[:],
                                 start=True, stop=True)
                st_row = small.tile([1, 2 * O], F32)
                nc.vector.tensor_copy(st_row[:], ps_st[:])

                cc_in = dram.tile([1, 2 * O], F32)
                cc_out = dram.tile([1, 2 * O], F32)
                nc.sync.dma_start(cc_in[:], st_row[:])
                if single_core or nocc:
                    nc.sync.dma_start(cc_out[:], cc_in[:])
                else:
                    nc.gpsimd.collective_compute(
                        "AllReduce", mybir.AluOpType.add,
                        replica_groups=[list(range(N_CORES))],
                        ins=[cc_in[:].opt()], outs=[cc_out[:].opt()])
                g_row = small.tile([1, 2 * O], F32)
                nc.sync.dma_start(g_row[:], cc_out[:])

                m_row = small.tile([1, 2 * O], F32)
                nc.vector.tensor_scalar_mul(m_row[:], g_row[:],
                                            1.0 / BN_COUNT)
                var_row = small.tile([1, O], F32)
                nc.vector.tensor_tensor(var_row[:], m_row[:, 0:O],
                                        m_row[:, 0:O], mybir.AluOpType.mult)
                nc.vector.tensor_sub(var_row[:], m_row[:, O:2 * O],
                                     var_row[:])
                eps_t = small.tile([1, 1], F32)
                nc.vector.memset(eps_t[:], BN_EPS)
                sd_row = small.tile([1, O], F32)
                nc.scalar.activation(sd_row[:], var_row[:], AF.Sqrt,
                                     bias=eps_t[:])
                scale_row = small.tile([1, O], F32)
                nc.vector.reciprocal(scale_row[:], sd_row[:])
                bias_row = small.tile([1, O], F32)
                nc.vector.scalar_tensor_tensor(bias_row[:], m_row[:, 0:O],
                                               -1.0, scale_row[:],
                                               mybir.AluOpType.mult,
                                               mybir.AluOpType.mult)

                # expand [1, O] -> [1, F] (repeat over b and t)
                row_sc = small.tile([1, F], F32)
                row_bi = small.tile([1, F], F32)
                sc_src = scale_row[:].unsqueeze(1).unsqueeze(2) \
                    .broadcast_to([1, BL, T, O])
                bi_src = bias_row[:].unsqueeze(1).unsqueeze(2) \
                    .broadcast_to([1, BL, T, O])
                nc.vector.tensor_copy(
                    row_sc[:].rearrange("p (b t o) -> p b t o",
                                        b=BL, t=T, o=O), sc_src)
                nc.vector.tensor_copy(
                    row_bi[:].rearrange("p (b t o) -> p b t o",
                                        b=BL, t=T, o=O), bi_src)

                scale_b = constp.tile([128, F], F32)
                bias_b = constp.tile([128, F], F32)
                nc.gpsimd.partition_broadcast(scale_b[:], row_sc[:])
                nc.gpsimd.partition_broadcast(bias_b[:], row_bi[:])

            out_v = out_d[:].rearrange("b (nt p) t o -> p nt b t o", p=128)
            for nt in range(NT):
                sl = out_sb[:, nt * F:(nt + 1) * F]
                if "norm" in stages and do_stats:
                    nc.vector.tensor_tensor(sl, sl, scale_b[:],
                                            mybir.AluOpType.mult)
                    nc.vector.tensor_tensor(sl, sl, bias_b[:],
                                            mybir.AluOpType.add)
                # gpsimd (SWDGE) casts bf16 -> f32 during the store
                nc.gpsimd.dma_start(
                    out_v[:, nt],
                    sl.rearrange("p (b t o) -> p b t o", b=BL, t=T, o=O))

    nc.compile()
    return nc


def _prep_inputs(x, cheb, theta):
    """Host-side shard/cast/layout prep (data movement only)."""
    cheb_bf = np.ascontiguousarray(cheb.astype(ml_dtypes.bfloat16))
    # block-diag theta: bd[(t*32+c), k*128 + t2*32 + o] = theta[k,c,o] if t==t2
    bd = np.zeros((128, K * 4 * O), dtype=ml_dtypes.bfloat16)
    th = theta.astype(ml_dtypes.bfloat16)
    for k in range(K):
        for t in range(4):
            bd[t * C:(t + 1) * C,
               k * 128 + t * O:(k * 128 + (t + 1) * O)] = th[k]
    in_maps = []
    for i in range(N_CORES):
        xs = x[i * BL:(i + 1) * BL]              # [BL, C, N, T]
        xs = np.ascontiguousarray(xs.transpose(0, 3, 1, 2))  # [BL, T, C, N]
        in_maps.append({
            "xt": xs.astype(ml_dtypes.bfloat16),
            "cheb": cheb_bf,
            "bd": bd,
        })
    return in_maps


def kernel(x, cheb, theta):
    x = np.asarray(x, dtype=np.float32)
    cheb = np.asarray(cheb, dtype=np.float32)
    theta = np.asarray(theta, dtype=np.float32)
    if "nc" not in _CACHE:
        _CACHE["nc"] = _build()
    nc = _CACHE["nc"]
    in_maps = _prep_inputs(x, cheb, theta)
    kw = {}
    if os.environ.get("BASS_KERNEL_TRACE") == "1":
        kw["trace"] = True
        kw["tmpdir"] = os.environ.get("BASS_KERNEL_TRACE_DIR") or None
    res = bass_utils.run_bass_kernel_spmd(nc, in_maps,
                                          core_ids=list(range(N_CORES)), **kw)
    global LAST_EXEC_NS
    LAST_EXEC_NS = res.exec_time_ns
    parts = []
    for i in range(N_CORES):
        o = res.results[i]["out"]                # [BL, N, T, O]
        parts.append(np.ascontiguousarray(o.transpose(0, 3, 1, 2)))
    return np.concatenate(parts, axis=0)


if __name__ == "__main__":
    rng = np.random.default_rng(0)
    x = rng.standard_normal((B, C, N, T)).astype(np.float32)
    cheb = rng.standard_normal((K, N, N)).astype(np.float32)
    theta = rng.standard_normal((K, C, O)).astype(np.float32)
    out = kernel(x, cheb, theta)
    print("out", out.shape, out.dtype, float(np.abs(out).mean()))



# revision 2
# speedup vs baseline: 1.4243x; 1.4243x over previous
"""GCN layer (Chebyshev) Trainium2 kernel, 8-core SPMD.

out = BatchNorm2d(einsum('kmn,bcmt,kco->bont', cheb, relu(x), theta))

Sharding: data-parallel over batch B=16 -> 2 batches/core, cheb+theta
replicated. BN batch stats combined with a tiny (256 B) AllReduce.

v2 changes vs baseline:
  - stage T pipelined per (b,tq): DMA -> relu -> 16 theta matmuls start
    after the first x tile lands (PE starts ~3us in, not ~40us).
  - out_sb kept in f32; output stored with plain HWDGE DMA (no gpsimd
    SWDGE cast path).
  - normalize: scale/bias rows broadcast across partitions via a PE
    outer product into PSUM; single pass of two DVE ops per half-tile.
  - straight-line `reps` replication of the FULL iteration (incl. the
    AllReduce) for honest slope timing; For_i `loop_reps` kept for
    body-only cross-checks.
"""

import contextlib
import os

import numpy as np
import ml_dtypes

import concourse.bass as bass
import concourse.bacc as bacc
import concourse.tile as tile
import concourse.mybir as mybir
import concourse.bass_utils as bass_utils

N_CORES = 8
B, C, N, T = 16, 32, 2048, 12
K, O = 3, 32
BL = B // N_CORES            # 2 batches per core
TQ = T // 4                  # 3 quads of 4 timesteps
MC = N // 128                # 16 m-chunks
NT = N // 128                # 16 n-tiles
CH = K * MC                  # 48 contraction chunks of 128
F = BL * T * O               # 768 free columns (b, t, o)
FH = F // 2                  # psum half
BN_EPS = 1e-5
BN_COUNT = B * N * T         # stats population per channel

BF16 = mybir.dt.bfloat16
F32 = mybir.dt.float32
AF = mybir.ActivationFunctionType
ALL_STAGES = frozenset({"theta", "big", "stats", "norm"})

_CACHE = {}


def _build(single_core=False, stages=ALL_STAGES, loop_reps=0, nocc=False,
           reps=1, tail_loop=False):
    nc = bacc.Bacc("TRN2", target_bir_lowering=False, debug=False,
                   num_devices=1 if single_core else N_CORES)
    xt = nc.dram_tensor("xt", [BL, T, C, N], BF16, kind="ExternalInput")
    # cheb pre-tiled on host: [nt, p(m in chunk), (k,mc) chunk, n(128)]
    cheb = nc.dram_tensor("cheb", [NT, 128, CH, 128], BF16,
                          kind="ExternalInput")
    bd = nc.dram_tensor("bd", [128, K * 4 * O], BF16, kind="ExternalInput")
    # output pre-tiled: [nt, p(n in tile), b, t, o]; host reassembles
    out_d = nc.dram_tensor("out", [NT, 128, BL, T, O], F32,
                           kind="ExternalOutput")

    with tile.TileContext(nc) as tc:
        with (
            tc.tile_pool(name="const", bufs=1) as constp,
            tc.tile_pool(name="xtlp", bufs=2) as xtlp,
            tc.tile_pool(name="xrp", bufs=2) as xrp,
            tc.tile_pool(name="wall", bufs=1) as wallp,
            tc.tile_pool(name="chebp", bufs=2) as chebp,
            tc.tile_pool(name="outp", bufs=1) as outp,
            tc.tile_pool(name="small", bufs=1) as small,
            tc.tile_pool(name="scratch", bufs=2) as scratch,
            tc.tile_pool(name="psw", bufs=3, space="PSUM") as psw,
            tc.tile_pool(name="psb", bufs=2, space="PSUM") as psb,
            tc.tile_pool(name="dram", bufs=1, space="DRAM") as dram,
        ):
            bd_s = constp.tile([128, K * 4 * O], BF16)
            nc.sync.dma_start(bd_s[:], bd[:])
            ones = small.tile([128, 1], F32)
            nc.vector.memset(ones[:], 1.0)
            ones_row = small.tile([1, 128], F32)
            nc.vector.memset(ones_row[:], 1.0)
            eps_t = small.tile([1, 1], F32)
            nc.vector.memset(eps_t[:], BN_EPS)

            # w[(k,mc) chunks x (b,t,o)]
            w_all = wallp.tile([128, CH * F], BF16)
            wv = w_all[:].rearrange("p (k mc b t o) -> p k mc b t o",
                                    k=K, mc=MC, b=BL, t=T, o=O)
            if "theta" not in stages and "big" in stages:
                nc.vector.memset(w_all[:], 0.5)
            out_sb = outp.tile([128, NT * F], F32)
            if "big" not in stages:
                nc.vector.memset(out_sb[:], 0.25)


            def emit_body(rep):
                stats = small.tile([128, 2 * O], F32, tag="stats")
                nc.vector.memset(stats[:], 0.0)

                # ---- stage T: pipelined load + relu + theta matmuls ----
                cnt = 0
                for b in range(BL):
                    for tq in range(TQ):
                        xtl = xtlp.tile([128, N], BF16, tag="xtl")
                        src = xt[b, tq * 4:(tq + 1) * 4].rearrange(
                            "t c m -> (t c) m")
                        nc.sync.dma_start(xtl[:], src)
                        if "theta" not in stages:
                            continue
                        xr = xrp.tile([128, N], BF16, tag="xr")
                        nc.scalar.activation(xr[:], xtl[:], AF.Relu)
                        for mc in range(MC):
                            pw = psw.tile([128, K * 4 * O], F32, tag="pw")
                            nc.tensor.matmul(
                                pw[:],
                                xr[:, mc * 128:(mc + 1) * 128],
                                bd_s[:], start=True, stop=True)
                            # pw free = (k, t', o); dest (k, t, o) slice
                            dest = wv[:, :, mc, b,
                                      tq * 4:(tq + 1) * 4, :]
                            srcv = pw[:].rearrange(
                                "p (k t o) -> p k t o", k=K, t=4, o=O)
                            if cnt % 2 == 0:
                                nc.vector.tensor_copy(dest, srcv)
                            else:
                                nc.scalar.copy(dest, srcv)
                            cnt += 1

                # ---- stage M: big matmul + stats partials ----
                for nt in range(NT):
                    cb = chebp.tile([128, CH, 128], BF16, tag="cb")
                    nc.sync.dma_start(cb[:], cheb[nt])
                    if "big" not in stages:
                        continue
                    po0 = psb.tile([128, FH], F32, tag="po0")
                    po1 = psb.tile([128, FH], F32, tag="po1")
                    n_ch = 0
                    for mc in range(MC):
                        for k in range(K):
                            ch = k * MC + mc
                            lhs = cb[:, ch, :]
                            first = n_ch == 0
                            last = n_ch == CH - 1
                            nc.tensor.matmul(
                                po0[:], lhs,
                                w_all[:, ch * F:ch * F + FH],
                                start=first, stop=last)
                            nc.tensor.matmul(
                                po1[:], lhs,
                                w_all[:, ch * F + FH:(ch + 1) * F],
                                start=first, stop=last)
                            n_ch += 1
                    sl = out_sb[:, nt * F:(nt + 1) * F]
                    nc.scalar.copy(sl[:, 0:FH], po0[:])
                    nc.scalar.copy(sl[:, FH:F], po1[:])
                    if "stats" in stages:
                        sq = scratch.tile([128, F], BF16, tag="sq")
                        nc.scalar.activation(sq[:, 0:FH], po0[:], AF.Square)
                        nc.scalar.activation(sq[:, FH:F], po1[:], AF.Square)
                        tmp_s = scratch.tile([128, O], F32, tag="tmp_s")
                        tmp_q = scratch.tile([128, O], F32, tag="tmp_q")
                        nc.vector.reduce_sum(
                            tmp_s[:],
                            sl.rearrange("p (b t o) -> p o b t",
                                         b=BL, t=T, o=O),
                            axis=mybir.AxisListType.XY)
                        nc.vector.reduce_sum(
                            tmp_q[:],
                            sq[:].rearrange("p (b t o) -> p o b t",
                                            b=BL, t=T, o=O),
                            axis=mybir.AxisListType.XY)
                        nc.vector.tensor_add(stats[:, 0:O], stats[:, 0:O],
                                             tmp_s[:])
                        nc.vector.tensor_add(stats[:, O:2 * O],
                                             stats[:, O:2 * O], tmp_q[:])

                return stats

            def emit_tail(rep, stats):
                # ---- stage S: finalize stats, AllReduce, normalize ----
                do_stats = "stats" in stages
                if do_stats:
                    ps_st = psw.tile([1, 2 * O], F32, tag="pw")
                    nc.tensor.matmul(ps_st[:], ones[:, 0:1], stats[:],
                                     start=True, stop=True)
                    st_row = small.tile([1, 2 * O], F32, tag="st_row")
                    nc.vector.tensor_copy(st_row[:], ps_st[:])

                    cc_in = dram.tile([1, 2 * O], F32, tag=f"cci{rep}")
                    cc_out = dram.tile([1, 2 * O], F32, tag=f"cco{rep}")
                    nc.sync.dma_start(cc_in[:], st_row[:])
                    if single_core or nocc:
                        nc.sync.dma_start(cc_out[:], cc_in[:])
                    else:
                        nc.gpsimd.collective_compute(
                            "AllReduce", mybir.AluOpType.add,
                            replica_groups=[list(range(N_CORES))],
                            ins=[cc_in[:].opt()], outs=[cc_out[:].opt()])
                    g_row = small.tile([1, 2 * O], F32, tag="g_row")
                    nc.sync.dma_start(g_row[:], cc_out[:])

                    m_row = small.tile([1, 2 * O], F32, tag="m_row")
                    nc.vector.tensor_scalar_mul(m_row[:], g_row[:],
                                                1.0 / BN_COUNT)
                    var_row = small.tile([1, O], F32, tag="var_row")
                    nc.vector.tensor_tensor(var_row[:], m_row[:, 0:O],
                                            m_row[:, 0:O],
                                            mybir.AluOpType.mult)
                    nc.vector.tensor_sub(var_row[:], m_row[:, O:2 * O],
                                         var_row[:])
                    sd_row = small.tile([1, O], F32, tag="sd_row")
                    nc.scalar.activation(sd_row[:], var_row[:], AF.Sqrt,
                                         bias=eps_t[:])
                    scale_row = small.tile([1, O], F32, tag="scale_row")
                    nc.vector.reciprocal(scale_row[:], sd_row[:])
                    bias_row = small.tile([1, O], F32, tag="bias_row")
                    nc.vector.scalar_tensor_tensor(bias_row[:], m_row[:, 0:O],
                                                   -1.0, scale_row[:],
                                                   mybir.AluOpType.mult,
                                                   mybir.AluOpType.mult)

                    # expand [1, O] -> [1, F] (repeat over b and t)
                    row_sc = small.tile([1, F], F32, tag="row_sc")
                    row_bi = small.tile([1, F], F32, tag="row_bi")
                    sc_src = scale_row[:].unsqueeze(1).unsqueeze(2) \
                        .broadcast_to([1, BL, T, O])
                    bi_src = bias_row[:].unsqueeze(1).unsqueeze(2) \
                        .broadcast_to([1, BL, T, O])
                    nc.vector.tensor_copy(
                        row_sc[:].rearrange("p (b t o) -> p b t o",
                                            b=BL, t=T, o=O), sc_src)
                    nc.vector.tensor_copy(
                        row_bi[:].rearrange("p (b t o) -> p b t o",
                                            b=BL, t=T, o=O), bi_src)

                    # broadcast across partitions via PE outer product
                    pb_s0 = psb.tile([128, FH], F32, tag="po0")
                    pb_s1 = psb.tile([128, FH], F32, tag="po1")
                    pb_b0 = psb.tile([128, FH], F32, tag="po0")
                    pb_b1 = psb.tile([128, FH], F32, tag="po1")
                    onr = ones_row[:]
                    nc.tensor.matmul(pb_s0[:], onr, row_sc[:, 0:FH],
                                     start=True, stop=True)
                    nc.tensor.matmul(pb_s1[:], onr, row_sc[:, FH:F],
                                     start=True, stop=True)
                    nc.tensor.matmul(pb_b0[:], onr, row_bi[:, 0:FH],
                                     start=True, stop=True)
                    nc.tensor.matmul(pb_b1[:], onr, row_bi[:, FH:F],
                                     start=True, stop=True)
                    scale_b = constp.tile([128, F], F32, tag="scale_b")
                    bias_b = constp.tile([128, F], F32, tag="bias_b")
                    nc.scalar.copy(scale_b[:, 0:FH], pb_s0[:])
                    nc.scalar.copy(scale_b[:, FH:F], pb_s1[:])
                    nc.scalar.copy(bias_b[:, 0:FH], pb_b0[:])
                    nc.scalar.copy(bias_b[:, FH:F], pb_b1[:])

                for nt in range(NT):
                    sl = out_sb[:, nt * F:(nt + 1) * F]
                    if "norm" in stages and do_stats:
                        nc.vector.tensor_tensor(sl, sl, scale_b[:],
                                                mybir.AluOpType.mult)
                        nc.vector.tensor_tensor(sl, sl, bias_b[:],
                                                mybir.AluOpType.add)
                    nc.sync.dma_start(
                        out_d[nt],
                        sl.rearrange("p (b t o) -> p b t o", b=BL, t=T, o=O))

            if loop_reps and tail_loop:
                # tail-only timing: loop the stats-finalize + normalize +
                # store tail with preset stats (mean 0 / var 1) and the
                # collective replaced by its local DMA hops (nocc).
                assert reps == 1 and nocc
                stats = small.tile([128, 2 * O], F32, tag="stats")
                nc.vector.memset(stats[:, 0:O], 0.0)
                nc.vector.memset(stats[:, O:2 * O], BN_COUNT / 128.0)
                with tc.For_i(0, loop_reps, 1):
                    emit_tail(0, stats)
            elif loop_reps:
                # collectives cannot sit inside control flow: loop the
                # body only, run the stats/normalize tail once after.
                assert reps == 1
                with tc.For_i(0, loop_reps, 1):
                    stats = emit_body(0)
                emit_tail(0, stats)
            else:
                for rep in range(reps):
                    stats = emit_body(rep)
                    emit_tail(rep, stats)

    nc.compile()
    return nc


def _prep_inputs(x, cheb, theta):
    """Host-side shard/cast/layout prep (data movement only)."""
    # pre-tile: cheb[k, mc*128+p, nt*128+n] -> ct[nt, p, k*MC+mc, n]
    ct = cheb.reshape(K, MC, 128, NT, 128).transpose(3, 2, 0, 1, 4) \
        .reshape(NT, 128, CH, 128)
    cheb_bf = np.ascontiguousarray(ct.astype(ml_dtypes.bfloat16))
    # block-diag theta: bd[(t*32+c), k*128 + t2*32 + o] = theta[k,c,o] if t==t2
    bd = np.zeros((128, K * 4 * O), dtype=ml_dtypes.bfloat16)
    th = theta.astype(ml_dtypes.bfloat16)
    for k in range(K):
        for t in range(4):
            bd[t * C:(t + 1) * C,
               k * 128 + t * O:(k * 128 + (t + 1) * O)] = th[k]
    in_maps = []
    for i in range(N_CORES):
        xs = x[i * BL:(i + 1) * BL]              # [BL, C, N, T]
        xs = np.ascontiguousarray(xs.transpose(0, 3, 1, 2))  # [BL, T, C, N]
        in_maps.append({
            "xt": xs.astype(ml_dtypes.bfloat16),
            "cheb": cheb_bf,
            "bd": bd,
        })
    return in_maps


def kernel(x, cheb, theta):
    x = np.asarray(x, dtype=np.float32)
    cheb = np.asarray(cheb, dtype=np.float32)
    theta = np.asarray(theta, dtype=np.float32)
    if "nc" not in _CACHE:
        _CACHE["nc"] = _build()
    nc = _CACHE["nc"]
    in_maps = _prep_inputs(x, cheb, theta)
    kw = {}
    if os.environ.get("BASS_KERNEL_TRACE") == "1":
        kw["trace"] = True
        kw["tmpdir"] = os.environ.get("BASS_KERNEL_TRACE_DIR") or None
    res = bass_utils.run_bass_kernel_spmd(nc, in_maps,
                                          core_ids=list(range(N_CORES)), **kw)
    global LAST_EXEC_NS
    LAST_EXEC_NS = res.exec_time_ns
    parts = []
    for i in range(N_CORES):
        o = res.results[i]["out"]                # [NT, 128, BL, T, O]
        o = o.reshape(N, BL, T, O).transpose(1, 3, 0, 2)   # [BL, O, N, T]
        parts.append(np.ascontiguousarray(o))
    return np.concatenate(parts, axis=0)


if __name__ == "__main__":
    rng = np.random.default_rng(0)
    x = rng.standard_normal((B, C, N, T)).astype(np.float32)
    cheb = rng.standard_normal((K, N, N)).astype(np.float32)
    theta = rng.standard_normal((K, C, O)).astype(np.float32)
    out = kernel(x, cheb, theta)
    print("out", out.shape, out.dtype, float(np.abs(out).mean()))
